# revision 26
# baseline (speedup 1.0000x reference)
"""Causal self-attention (B=4, S=2048, D=2048, H=16) on 8 Trainium2 cores.

Sharding: core c -> (batch b = c//2, head-half = c%2, i.e. 8 of 16 heads).
Megatron-style: Wq/Wk/Wv column-parallel (8 heads' rows), Wo row-parallel
(matching 1024 columns).  Each core emits a partial (S, D) output for its
batch; host sums the two half partials per batch and adds bo_eff
(bo + Wo @ bv -- the V bias folds out of the device program because
softmax rows sum to one).

v2: all-bf16, fully SBUF-resident (no DRAM spills of Q/K/V/ctx).
  Stage 1 (two 4-head passes): QKV projections -> per-(head,s-chunk)
    bf16 SBUF tiles.  Q/K bias-add on DVE (per-partition scalar).
  Stage 2 (qc-outer, head-inner, 2-deep software pipeline):
    scores^T tile [k,q] = K_chunk.T @ Q  (restricted to causal q-range)
    es = Exp(scale*S^T) on ACT -> bf16; 128-wide diagonal triangle masked
    in place with gpsimd affine_select; row-sum accumulated on DVE.
    ctx^T [dv,q] = sum_k V_chunk.T @ es  (PSUM accumulation)
    denominator: ones-column matmul partition-reduce -> [1,q] PSUM,
    DVE reciprocal -> ones-row matmul broadcast -> [128,q] PSUM ->
    DVE copy to SBUF -> DVE mul normalizes ctx to bf16.
    After each qc's 8 heads: out-projection for that q-chunk
    (stationary ctx slices, moving Wo e-chunks, 8-head PSUM chain).
"""

import math

import numpy as np
import ml_dtypes

import concourse.bass as bass
import concourse.mybir as mybir
from concourse.bass_utils import run_bass_kernel_spmd
from concourse.tile import TileContext

B, S, D, H = 4, 2048, 2048, 16
DK = 128
NCORES = 8
HPC = H // 2          # 8 heads per core
MLOC = HPC * DK       # 1024 local head dims
NSC = S // 512        # 4 s/q chunks of 512
SCALE = 1.0 / math.sqrt(DK)

F32 = mybir.dt.float32
F32R = mybir.dt.float32r
BF16 = mybir.dt.bfloat16
AF = mybir.ActivationFunctionType
BF16_NP = ml_dtypes.bfloat16


def split_excess_waits(nc, max_waits=1):
    """walrus in this container accepts at most one sem-wait per instruction;
    move excess waits onto wait-only EventSemaphore insts inserted before."""
    ctr = 0
    for f in nc.m.functions:
        for bb in f.blocks:
            new = []
            changed = False
            for inst in bb.instructions:
                si = inst.sync_info
                if si is not None and si.on_wait and len(si.on_wait) > max_waits:
                    changed = True
                    waits = list(si.on_wait)
                    for w in waits[:-max_waits]:
                        ctr += 1
                        ev = mybir.InstEventSemaphore(
                            name=f"waitsplit-{ctr}", ins=[], outs=[],
                            sync_info=mybir.SyncInfo(on_wait=[w], on_update=[]))
                        ev.engine = inst.engine
                        new.append(ev)
                    si.on_wait = waits[-max_waits:]
                new.append(inst)
            if changed:
                bb.instructions = new
    return ctr


def build_nc(seq=S):
    assert seq % 512 == 0
    nsc = seq // 512

    nc = bass.Bass()
    xt = nc.declare_dram_parameter("xt", [D, seq], BF16, isOutput=False)
    wqt = nc.declare_dram_parameter("wqt", [D, MLOC], BF16, isOutput=False)
    wkt = nc.declare_dram_parameter("wkt", [D, MLOC], BF16, isOutput=False)
    wvt = nc.declare_dram_parameter("wvt", [D, MLOC], BF16, isOutput=False)
    wot = nc.declare_dram_parameter("wot", [MLOC, D], BF16, isOutput=False)
    bqt = nc.declare_dram_parameter("bqt", [DK, HPC], F32, isOutput=False)
    bkt = nc.declare_dram_parameter("bkt", [DK, HPC], F32, isOutput=False)
    ones = nc.declare_dram_parameter("ones", [128, 128], F32R, isOutput=False)
    out = nc.declare_dram_parameter("out", [seq, D], F32, isOutput=True)

    xt_r = xt.rearrange("(dc p) s -> p dc s", p=128)      # [128, 16, seq]
    wqt_r = wqt.rearrange("(dc p) m -> p dc m", p=128)    # [128, 16, 1024]
    wkt_r = wkt.rearrange("(dc p) m -> p dc m", p=128)
    wvt_r = wvt.rearrange("(dc p) m -> p dc m", p=128)
    wot_r = wot.rearrange("(hc p) e -> p hc e", p=128)    # [128, 8, 2048]

    with TileContext(nc) as tc:
        with tc.tile_pool(name="const", bufs=1) as cpool, \
             tc.tile_pool(name="qkv", bufs=1) as qkvpool:
            bq_sb = cpool.tile([DK, HPC], F32, name="bqsb")
            nc.sync.dma_start(out=bq_sb[:], in_=bqt[:])
            bk_sb = cpool.tile([DK, HPC], F32, name="bksb")
            nc.sync.dma_start(out=bk_sb[:], in_=bkt[:])
            ones_b = cpool.tile([128, 512], BF16, name="onesb")
            nc.vector.memset(ones_b[:], 1.0)
            onesrow = cpool.tile([1, 128], BF16, name="onesrow")
            nc.vector.memset(onesrow[:], 1.0)
            onescol = cpool.tile([128, 1], F32R, name="onescol")
            nc.sync.dma_start(out=onescol[:], in_=ones[:, 0:1])

            # per-(head, s-chunk) Q/K tiles [dk, 512]; per-(pass, s-chunk)
            # V tiles [kpos, 4kc, 512dv] -> fine-grained stage1->2 deps
            q_t = [[qkvpool.tile([128, 512], BF16, name=f"q{h}_{sc}")
                    for sc in range(nsc)] for h in range(HPC)]
            k_t = [[qkvpool.tile([128, 512], BF16, name=f"k{h}_{sc}")
                    for sc in range(nsc)] for h in range(HPC)]
            v_t = [[qkvpool.tile([128, 4, 512], BF16, name=f"v{p_}_{sc}")
                    for sc in range(nsc)] for p_ in range(2)]

            # ---------------- Stage 1: QKV projections ----------------
            with tc.tile_pool(name="s1w", bufs=1) as wpool, \
                 tc.tile_pool(name="s1x", bufs=2) as xpool, \
                 tc.tile_pool(name="psqk", bufs=4, space="PSUM") as qkp, \
                 tc.tile_pool(name="psv", bufs=2, space="PSUM") as vps, \
                 tc.tile_pool(name="pswarm", bufs=1, space="PSUM") as wmp:
                # PE warm-up during initial DMA: keeps HAM busy so real
                # matmuls start at K=8/8 (never read back).
                wp = wmp.tile([128, 512], F32, tag="warm")
                for i in range(56):
                    nc.tensor.matmul(wp[:], ones_b[:, 0:128], ones_b[:],
                                     start=(i == 0), stop=(i == 55))
                for p_ in range(2):           # head-half pass: heads 4p..4p+3
                    wq_sb = wpool.tile([128, 16, 512], BF16, tag="wq")
                    nc.sync.dma_start(out=wq_sb[:], in_=wqt_r[:, :, p_*512:(p_+1)*512])
                    xin0 = None
                    if p_ == 0:   # prefetch first x chunk before wk/wv so the
                        xin0 = xpool.tile([128, 16, 512], BF16, tag="xin")
                        nc.sync.dma_start(out=xin0[:], in_=xt_r[:, :, 0:512])
                    wk_sb = wpool.tile([128, 16, 512], BF16, tag="wk")
                    nc.sync.dma_start(out=wk_sb[:], in_=wkt_r[:, :, p_*512:(p_+1)*512])
                    wv_sb = wpool.tile([128, 16, 512], BF16, tag="wv")
                    nc.sync.dma_start(out=wv_sb[:], in_=wvt_r[:, :, p_*512:(p_+1)*512])
                    for sc in range(nsc):
                        if p_ == 0 and sc == 0:
                            xin = xin0
                        else:
                            xin = xpool.tile([128, 16, 512], BF16, tag="xin")
                            nc.sync.dma_start(out=xin[:], in_=xt_r[:, :, sc*512:(sc+1)*512])
                        for hh in range(4):
                            h = p_ * 4 + hh
                            qps = qkp.tile([128, 512], F32, tag="qk")
                            for dc in range(16):
                                nc.tensor.matmul(
                                    qps[:], wq_sb[:, dc, hh*128:(hh+1)*128],
                                    xin[:, dc, :], start=(dc == 0), stop=(dc == 15))
                            nc.vector.tensor_scalar_add(
                                q_t[h][sc][:], qps[:], bq_sb[:, h:h+1])
                            kps = qkp.tile([128, 512], F32, tag="qk")
                            for dc in range(16):
                                nc.tensor.matmul(
                                    kps[:], wk_sb[:, dc, hh*128:(hh+1)*128],
                                    xin[:, dc, :], start=(dc == 0), stop=(dc == 15))
                            nc.vector.tensor_scalar_add(
                                k_t[h][sc][:], kps[:], bk_sb[:, h:h+1])
                        for kc in range(4):
                            vp = vps.tile([128, 512], F32, tag="v")
                            for dc in range(16):
                                nc.tensor.matmul(
                                    vp[:], xin[:, dc, kc*128:(kc+1)*128],
                                    wv_sb[:, dc, :], start=(dc == 0), stop=(dc == 15))
                            nc.vector.tensor_copy(v_t[p_][sc][:, kc, :], vp[:])

            # ------------- Stage 2: attention + out-projection -------------
            with tc.tile_pool(name="s2es", bufs=2) as espool, \
                 tc.tile_pool(name="s2ctx", bufs=2) as ctxpool, \
                 tc.tile_pool(name="s2wo", bufs=1) as wopool, \
                 tc.tile_pool(name="s2acc", bufs=2) as accpool, \
                 tc.tile_pool(name="s2rcp", bufs=2) as rcppool, \
                 tc.tile_pool(name="s2rcpb", bufs=2) as rcpbpool, \
                 tc.tile_pool(name="s2rcb", bufs=2) as rcbpool, \
                 tc.tile_pool(name="s2o", bufs=2) as opool, \
                 tc.tile_pool(name="psp", bufs=2, space="PSUM") as psp, \
                 tc.tile_pool(name="pcd", bufs=2, space="PSUM") as pcd, \
                 tc.tile_pool(name="pdn", bufs=1, space="PSUM") as pdn, \
                 tc.tile_pool(name="prb", bufs=1, space="PSUM") as prb, \
                 tc.tile_pool(name="pop", bufs=2, space="PSUM") as pop:
                wo_sb = []
                for ec in range(4):
                    w = wopool.tile([128, HPC, 512], BF16, name=f"wo{ec}")
                    nc.sync.dma_start(out=w[:], in_=wot_r[:, :, ec*512:(ec+1)*512])
                    wo_sb.append(w)

                ctx_sb = {}     # qc -> [128 dv, HPC, 512 q] bf16
                st = {}         # unit -> intermediates

                def emit_score_kc(u, kc):
                    """scores matmul + exp + causal mask for one k-chunk."""
                    qc, h = u
                    es = st[u]
                    j = kc - 4 * qc
                    lo = 128 * j if j > 0 else 0
                    sp = psp.tile([128, 512], F32, tag="sp")
                    nc.tensor.matmul(
                        sp[:, lo:], k_t[h][kc // 4][:, (kc % 4)*128:(kc % 4+1)*128],
                        q_t[h][qc][:, lo:], start=True, stop=True)
                    nc.scalar.activation(es[:, kc, lo:], sp[:, lo:],
                                         AF.Exp, bias=0.0, scale=SCALE)
                    if j >= 0:   # diagonal 128-block: causal triangle
                        nc.gpsimd.affine_select(
                            out=es[:, kc, lo:lo+128], in_=es[:, kc, lo:lo+128],
                            compare_op=mybir.AluOpType.is_ge, fill=0.0,
                            base=0, pattern=[[1, 128]], channel_multiplier=-1)

                def emit_ctx_kc(u, kc, cp):
                    """ctx accumulation for one k-chunk."""
                    qc, h = u
                    es = st[u]
                    nk = 4 * qc + 4
                    p_, hh = h // 4, h % 4
                    j = kc - 4 * qc
                    lo = 128 * j if j > 0 else 0
                    nc.tensor.matmul(
                        cp[:, lo:], v_t[p_][kc // 4][:, kc % 4, hh*128:(hh+1)*128],
                        es[:, kc, lo:], start=(kc == 0), stop=(kc == nk - 1))

                def emit_dn(u, dn):
                    """denominator: batched ones-column matmuls (holds the
                    PSUM bank only briefly at the end of the step)."""
                    qc, h = u
                    es = st[u]
                    nk = 4 * qc + 4
                    for kc in range(nk):
                        j = kc - 4 * qc
                        lo = 128 * j if j > 0 else 0
                        nc.tensor.matmul(
                            dn[:, lo:], ones_b[:, 0:1], es[:, kc, lo:],
                            start=(kc == 0), stop=(kc == nk - 1))

                def emit_norm(u):
                    """reciprocal of the denominator row (serial DVE)."""
                    cp, dn = st[u]
                    rcp = rcppool.tile([1, 512], F32, tag="rcp")
                    nc.vector.reciprocal(rcp[:], dn[:])
                    rcpb = rcpbpool.tile([1, 512], BF16, tag="rcpb")
                    nc.vector.tensor_copy(rcpb[:], rcp[:])
                    st[u] = (cp, rcpb)

                def emit_finish(u):
                    """broadcast reciprocal over partitions + normalize."""
                    qc, h = u
                    cp, rcpb = st.pop(u)
                    rb = prb.tile([128, 512], F32, tag="rb")
                    nc.tensor.matmul(rb[:], onesrow[:], rcpb[:],
                                     start=True, stop=True)
                    rbs = rcbpool.tile([128, 512], F32, tag="rbs")
                    nc.scalar.copy(rbs[:], rb[:])
                    if qc not in ctx_sb:
                        ctx_sb[qc] = ctxpool.tile([128, HPC, 512], BF16,
                                                  tag="ctx", name=f"ctx{qc}")
                    nc.vector.tensor_mul(ctx_sb[qc][:, h, :], cp[:], rbs[:])

                def outproj(qc):
                    ctx = ctx_sb.pop(qc)
                    for ss in range(4):
                        for ec in range(4):
                            op = pop.tile([128, 512], F32, tag="op")
                            for h in range(HPC):
                                nc.tensor.matmul(
                                    op[:], ctx[:, h, ss*128:(ss+1)*128],
                                    wo_sb[ec][:, h, :],
                                    start=(h == 0), stop=(h == HPC - 1))
                            o_sb = opool.tile([128, 512], F32, tag="o")
                            if ec % 2 == 0:
                                nc.scalar.copy(o_sb[:], op[:])
                            else:
                                nc.vector.tensor_copy(o_sb[:], op[:])
                            nc.sync.dma_start(
                                out=out[qc*512+ss*128:qc*512+(ss+1)*128,
                                        ec*512:(ec+1)*512],
                                in_=o_sb[:])

                # Pair short (qc=0/1) units with long (qc=3/2) ones so the
                # serial per-unit reciprocal always hides under a long
                # neighbour's matmul stream; interleave score/ctx matmuls
                # kc-by-kc so the PE never throttles to ACT's exp pace.
                units = []
                for h in range(HPC):
                    units += [(0, h), (3, h)]
                for h in range(HPC):
                    units += [(1, h), (2, h)]
                ndone = {qc: 0 for qc in range(nsc)}
                for i in range(len(units) + 2):
                    cur = units[i] if i < len(units) else None
                    prv = units[i - 1] if 0 <= i - 1 < len(units) else None
                    if cur is not None:
                        st[cur] = espool.tile([128, 16, 512], BF16, tag="es",
                                              name=f"es_{i}")
                    if prv is not None:
                        es_p = st[prv]
                        cp = pcd.tile([128, 512], F32, tag="cd")
                        dn = pdn.tile([1, 512], F32, tag="dn")
                    nk_c = 4 * cur[0] + 4 if cur is not None else 0
                    nk_p = 4 * prv[0] + 4 if prv is not None else 0
                    for kc in range(max(nk_c, nk_p)):
                        if kc < nk_c:
                            emit_score_kc(cur, kc)
                        if kc < nk_p:
                            emit_ctx_kc(prv, kc, cp)
                    if prv is not None:
                        emit_dn(prv, dn)
                        st[prv] = (cp, dn)
                        emit_norm(prv)
                    if 0 <= i - 2 < len(units):
                        u2 = units[i - 2]
                        emit_finish(u2)
                        ndone[u2[0]] += 1
                        if ndone[u2[0]] == HPC:
                            outproj(u2[0])
    split_excess_waits(nc)
    return nc


_NC_CACHE = {}


def _get_nc(seq):
    if seq not in _NC_CACHE:
        _NC_CACHE[seq] = build_nc(seq)
    return _NC_CACHE[seq]


def make_in_maps(x, Wq, bq, Wk, bk, Wv, bv, Wo, bo, seq=S, nb=B):
    f32 = np.float32
    in_maps = []
    for c in range(NCORES):
        b = c // 2
        half = c % 2
        sl = slice(half * MLOC, (half + 1) * MLOC)
        in_maps.append({
            "xt": np.ascontiguousarray(x[b].T).astype(BF16_NP),
            "wqt": np.ascontiguousarray(Wq[sl, :].T).astype(BF16_NP),
            "wkt": np.ascontiguousarray(Wk[sl, :].T).astype(BF16_NP),
            "wvt": np.ascontiguousarray(Wv[sl, :].T).astype(BF16_NP),
            "wot": np.ascontiguousarray(Wo[:, sl].T).astype(BF16_NP),
            "bqt": np.ascontiguousarray(bq[sl].reshape(HPC, DK).T, dtype=f32),
            "bkt": np.ascontiguousarray(bk[sl].reshape(HPC, DK).T, dtype=f32),
            "ones": np.ones((128, 128), dtype=f32),
        })
    return in_maps


def run(inputs, trace=False, trace_kwargs=None):
    x = np.asarray(inputs["x"], dtype=np.float32)
    nb, seq, d = x.shape
    nc = _get_nc(seq)
    Wo = np.asarray(inputs["Wo"], dtype=np.float32)
    bv = np.asarray(inputs["bv"], dtype=np.float32)
    bo = np.asarray(inputs["bo"], dtype=np.float32)
    in_maps = make_in_maps(
        x, np.asarray(inputs["Wq"]), np.asarray(inputs["bq"]),
        np.asarray(inputs["Wk"]), np.asarray(inputs["bk"]),
        np.asarray(inputs["Wv"]), bv, Wo, bo, seq=seq, nb=nb)
    res = run_bass_kernel_spmd(nc, in_maps, list(range(NCORES)), trace=trace,
                               **(trace_kwargs or {}))
    # V-bias folds out of the device program: ctx = attn@V + bv (rows of
    # attn sum to 1), so its out-projection contribution is bv @ Wo.T.
    bo_eff = bo + Wo @ bv
    out = np.empty((nb, seq, d), dtype=np.float32)
    for b in range(nb):
        out[b] = res.results[2*b]["out"] + res.results[2*b+1]["out"] + bo_eff
    return out, res


def kernel(**inputs):
    out, _ = run(inputs, trace=False)
    return out


# revision 31
# speedup vs baseline: 1.1324x; 1.1324x over previous
"""Causal self-attention (B=4, S=2048, D=2048, H=16) on 8 Trainium2 cores.

Sharding: core c -> (batch b = c//2, head-half = c%2, i.e. 8 of 16 heads).
Megatron-style: Wq/Wk/Wv column-parallel (8 heads' rows), Wo row-parallel
(matching 1024 columns).  Each core emits a partial (S, D) output for its
batch; host sums the two half partials per batch and adds bo_eff
(bo + Wo @ bv -- the V bias folds out of the device program because
softmax rows sum to one).

v2: all-bf16, fully SBUF-resident (no DRAM spills of Q/K/V/ctx).
  Stage 1 (two 4-head passes): QKV projections -> per-(head,s-chunk)
    bf16 SBUF tiles.  Q/K bias-add on DVE (per-partition scalar).
  Stage 2 (qc-outer, head-inner, 2-deep software pipeline):
    scores^T tile [k,q] = K_chunk.T @ Q  (restricted to causal q-range)
    es = Exp(scale*S^T) on ACT -> bf16; 128-wide diagonal triangle masked
    in place with gpsimd affine_select; row-sum accumulated on DVE.
    ctx^T [dv,q] = sum_k V_chunk.T @ es  (PSUM accumulation)
    denominator: ones-column matmul partition-reduce -> [1,q] PSUM,
    DVE reciprocal -> ones-row matmul broadcast -> [128,q] PSUM ->
    DVE copy to SBUF -> DVE mul normalizes ctx to bf16.
    After each qc's 8 heads: out-projection for that q-chunk
    (stationary ctx slices, moving Wo e-chunks, 8-head PSUM chain).
"""

import math

import numpy as np
import ml_dtypes

import concourse.bass as bass
import concourse.mybir as mybir
from concourse.bass_utils import run_bass_kernel_spmd
from concourse.tile import TileContext

B, S, D, H = 4, 2048, 2048, 16
DK = 128
NCORES = 8
HPC = H // 2          # 8 heads per core
MLOC = HPC * DK       # 1024 local head dims
NSC = S // 512        # 4 s/q chunks of 512
SCALE = 1.0 / math.sqrt(DK)

F32 = mybir.dt.float32
F32R = mybir.dt.float32r
BF16 = mybir.dt.bfloat16
AF = mybir.ActivationFunctionType
BF16_NP = ml_dtypes.bfloat16


def split_excess_waits(nc, max_waits=1):
    """walrus in this container accepts at most one sem-wait per instruction;
    move excess waits onto wait-only EventSemaphore insts inserted before."""
    ctr = 0
    for f in nc.m.functions:
        for bb in f.blocks:
            new = []
            changed = False
            for inst in bb.instructions:
                si = inst.sync_info
                if si is not None and si.on_wait and len(si.on_wait) > max_waits:
                    changed = True
                    waits = list(si.on_wait)
                    for w in waits[:-max_waits]:
                        ctr += 1
                        ev = mybir.InstEventSemaphore(
                            name=f"waitsplit-{ctr}", ins=[], outs=[],
                            sync_info=mybir.SyncInfo(on_wait=[w], on_update=[]))
                        ev.engine = inst.engine
                        new.append(ev)
                    si.on_wait = waits[-max_waits:]
                new.append(inst)
            if changed:
                bb.instructions = new
    return ctr


def build_nc(seq=S):
    assert seq % 512 == 0
    nsc = seq // 512

    nc = bass.Bass()
    xt = nc.declare_dram_parameter("xt", [D, seq], BF16, isOutput=False)
    wqt = nc.declare_dram_parameter("wqt", [D, MLOC], BF16, isOutput=False)
    wkt = nc.declare_dram_parameter("wkt", [D, MLOC], BF16, isOutput=False)
    wvt = nc.declare_dram_parameter("wvt", [D, MLOC], BF16, isOutput=False)
    wot = nc.declare_dram_parameter("wot", [MLOC, D], BF16, isOutput=False)
    bqt = nc.declare_dram_parameter("bqt", [DK, HPC], F32, isOutput=False)
    bkt = nc.declare_dram_parameter("bkt", [DK, HPC], F32, isOutput=False)
    ones = nc.declare_dram_parameter("ones", [128, 128], F32R, isOutput=False)
    out = nc.declare_dram_parameter("out", [seq, D], F32, isOutput=True)

    xt_r = xt.rearrange("(dc p) s -> p dc s", p=128)      # [128, 16, seq]
    wqt_r = wqt.rearrange("(dc p) m -> p dc m", p=128)    # [128, 16, 1024]
    wkt_r = wkt.rearrange("(dc p) m -> p dc m", p=128)
    wvt_r = wvt.rearrange("(dc p) m -> p dc m", p=128)
    wot_r = wot.rearrange("(hc p) e -> p hc e", p=128)    # [128, 8, 2048]

    with TileContext(nc) as tc:
        with tc.tile_pool(name="const", bufs=1) as cpool, \
             tc.tile_pool(name="qkv", bufs=1) as qkvpool:
            bq_sb = cpool.tile([DK, HPC], F32, name="bqsb")
            nc.sync.dma_start(out=bq_sb[:], in_=bqt[:])
            bk_sb = cpool.tile([DK, HPC], F32, name="bksb")
            nc.sync.dma_start(out=bk_sb[:], in_=bkt[:])
            ones_b = cpool.tile([128, 512], BF16, name="onesb")
            nc.vector.memset(ones_b[:], 1.0)
            onesrow = cpool.tile([1, 128], BF16, name="onesrow")
            nc.vector.memset(onesrow[:], 1.0)
            onescol = cpool.tile([128, 1], F32R, name="onescol")
            nc.sync.dma_start(out=onescol[:], in_=ones[:, 0:1])

            # per-(head, s-chunk) Q/K tiles [dk, 512]; per-(pass, s-chunk)
            # V tiles [kpos, 4kc, 512dv] -> fine-grained stage1->2 deps
            q_t = [[qkvpool.tile([128, 512], BF16, name=f"q{h}_{sc}")
                    for sc in range(nsc)] for h in range(HPC)]
            k_t = [[qkvpool.tile([128, 512], BF16, name=f"k{h}_{sc}")
                    for sc in range(nsc)] for h in range(HPC)]
            v_t = [[qkvpool.tile([128, 4, 512], BF16, name=f"v{p_}_{sc}")
                    for sc in range(nsc)] for p_ in range(2)]

            # ---------------- Stage 1: QKV projections ----------------
            with tc.tile_pool(name="s1w", bufs=1) as wpool, \
                 tc.tile_pool(name="s1x", bufs=2) as xpool, \
                 tc.tile_pool(name="psqk", bufs=4, space="PSUM") as qkp, \
                 tc.tile_pool(name="psv", bufs=2, space="PSUM") as vps, \
                 tc.tile_pool(name="pswarm", bufs=1, space="PSUM") as wmp:
                # PE warm-up during initial DMA: keeps HAM busy so real
                # matmuls start at K=8/8 (never read back).
                wp = wmp.tile([128, 512], F32, tag="warm")
                for i in range(56):
                    nc.tensor.matmul(wp[:], ones_b[:, 0:128], ones_b[:],
                                     start=(i == 0), stop=(i == 55))
                for p_ in range(2):           # head-half pass: heads 4p..4p+3
                    wq_sb = wpool.tile([128, 16, 512], BF16, tag="wq")
                    nc.sync.dma_start(out=wq_sb[:], in_=wqt_r[:, :, p_*512:(p_+1)*512])
                    xin0 = None
                    if p_ == 0:   # prefetch first x chunk before wk/wv so the
                        xin0 = xpool.tile([128, 16, 512], BF16, tag="xin")
                        nc.sync.dma_start(out=xin0[:], in_=xt_r[:, :, 0:512])
                    wk_sb = wpool.tile([128, 16, 512], BF16, tag="wk")
                    nc.sync.dma_start(out=wk_sb[:], in_=wkt_r[:, :, p_*512:(p_+1)*512])
                    wv_sb = wpool.tile([128, 16, 512], BF16, tag="wv")
                    nc.sync.dma_start(out=wv_sb[:], in_=wvt_r[:, :, p_*512:(p_+1)*512])
                    for sc in range(nsc):
                        if p_ == 0 and sc == 0:
                            xin = xin0
                        else:
                            xin = xpool.tile([128, 16, 512], BF16, tag="xin")
                            nc.sync.dma_start(out=xin[:], in_=xt_r[:, :, sc*512:(sc+1)*512])
                        for hh in range(4):
                            h = p_ * 4 + hh
                            qps = qkp.tile([128, 512], F32, tag="qk")
                            for dc in range(16):
                                nc.tensor.matmul(
                                    qps[:], wq_sb[:, dc, hh*128:(hh+1)*128],
                                    xin[:, dc, :], start=(dc == 0), stop=(dc == 15))
                            nc.vector.tensor_scalar_add(
                                q_t[h][sc][:], qps[:], bq_sb[:, h:h+1])
                            kps = qkp.tile([128, 512], F32, tag="qk")
                            for dc in range(16):
                                nc.tensor.matmul(
                                    kps[:], wk_sb[:, dc, hh*128:(hh+1)*128],
                                    xin[:, dc, :], start=(dc == 0), stop=(dc == 15))
                            nc.vector.tensor_scalar_add(
                                k_t[h][sc][:], kps[:], bk_sb[:, h:h+1])
                        for kc in range(4):
                            vp = vps.tile([128, 512], F32, tag="v")
                            for dc in range(16):
                                nc.tensor.matmul(
                                    vp[:], xin[:, dc, kc*128:(kc+1)*128],
                                    wv_sb[:, dc, :], start=(dc == 0), stop=(dc == 15))
                            nc.vector.tensor_copy(v_t[p_][sc][:, kc, :], vp[:])

            # ------------- Stage 2: attention + out-projection -------------
            with tc.tile_pool(name="s2es", bufs=2) as espool, \
                 tc.tile_pool(name="s2ctx", bufs=2) as ctxpool, \
                 tc.tile_pool(name="s2wo", bufs=1) as wopool, \
                 tc.tile_pool(name="s2dns", bufs=2) as dnspool, \
                 tc.tile_pool(name="s2rcp", bufs=2) as rcppool, \
                 tc.tile_pool(name="s2rcpb", bufs=2) as rcpbpool, \
                 tc.tile_pool(name="s2rcb", bufs=2) as rcbpool, \
                 tc.tile_pool(name="s2o", bufs=2) as opool, \
                 tc.tile_pool(name="psp", bufs=2, space="PSUM") as psp, \
                 tc.tile_pool(name="pcd", bufs=2, space="PSUM") as pcd, \
                 tc.tile_pool(name="pdn", bufs=1, space="PSUM") as pdn, \
                 tc.tile_pool(name="prb", bufs=1, space="PSUM") as prb, \
                 tc.tile_pool(name="pop", bufs=2, space="PSUM") as pop:
                wo_sb = []
                for ec in range(4):
                    w = wopool.tile([128, HPC, 512], BF16, name=f"wo{ec}")
                    nc.sync.dma_start(out=w[:], in_=wot_r[:, :, ec*512:(ec+1)*512])
                    wo_sb.append(w)

                ctx_sb = {}     # qc -> [128 dv, HPC, 512 q] bf16
                st = {}         # unit -> intermediates

                def emit_score_kc(u, kc):
                    """scores matmul + exp + causal mask for one k-chunk."""
                    qc, h = u
                    es = st[u]
                    j = kc - 4 * qc
                    lo = 128 * j if j > 0 else 0
                    sp = psp.tile([128, 512], F32, tag="sp")
                    nc.tensor.matmul(
                        sp[:, lo:], k_t[h][kc // 4][:, (kc % 4)*128:(kc % 4+1)*128],
                        q_t[h][qc][:, lo:], start=True, stop=True)
                    nc.scalar.activation(es[:, kc, lo:], sp[:, lo:],
                                         AF.Exp, bias=0.0, scale=SCALE)
                    if j >= 0:   # diagonal 128-block: causal triangle
                        nc.gpsimd.affine_select(
                            out=es[:, kc, lo:lo+128], in_=es[:, kc, lo:lo+128],
                            compare_op=mybir.AluOpType.is_ge, fill=0.0,
                            base=0, pattern=[[1, 128]], channel_multiplier=-1)

                def emit_ctx_kc(u, kc, cp):
                    """ctx accumulation for one k-chunk."""
                    qc, h = u
                    es = st[u]
                    nk = 4 * qc + 4
                    p_, hh = h // 4, h % 4
                    j = kc - 4 * qc
                    lo = 128 * j if j > 0 else 0
                    nc.tensor.matmul(
                        cp[:, lo:], v_t[p_][kc // 4][:, kc % 4, hh*128:(hh+1)*128],
                        es[:, kc, lo:], start=(kc == 0), stop=(kc == nk - 1))

                def emit_dn(u, dn):
                    """denominator: batched ones-column matmuls (holds the
                    PSUM bank only briefly at the end of the step)."""
                    qc, h = u
                    es = st[u]
                    nk = 4 * qc + 4
                    for kc in range(nk):
                        j = kc - 4 * qc
                        lo = 128 * j if j > 0 else 0
                        nc.tensor.matmul(
                            dn[:, lo:], ones_b[:, 0:1], es[:, kc, lo:],
                            start=(kc == 0), stop=(kc == nk - 1))

                def emit_norm(u):
                    """drain denominator to SBUF (frees the PSUM bank fast),
                    then serial reciprocal + bf16 cast on DVE."""
                    cp, dn = st[u]
                    dns = dnspool.tile([1, 512], F32, tag="dns")
                    nc.vector.tensor_copy(dns[:], dn[:])
                    rcp = rcppool.tile([1, 512], F32, tag="rcp")
                    nc.vector.reciprocal(rcp[:], dns[:])
                    rcpb = rcpbpool.tile([1, 512], BF16, tag="rcpb")
                    nc.vector.tensor_copy(rcpb[:], rcp[:])
                    st[u] = (cp, rcpb)

                def emit_finish(u):
                    """broadcast reciprocal over partitions + normalize."""
                    qc, h = u
                    cp, rcpb = st.pop(u)
                    rb = prb.tile([128, 512], F32, tag="rb")
                    nc.tensor.matmul(rb[:], onesrow[:], rcpb[:],
                                     start=True, stop=True)
                    rbs = rcbpool.tile([128, 512], F32, tag="rbs")
                    nc.scalar.copy(rbs[:], rb[:])
                    if qc not in ctx_sb:
                        ctx_sb[qc] = ctxpool.tile([128, HPC, 512], BF16,
                                                  tag="ctx", name=f"ctx{qc}")
                    nc.vector.tensor_mul(ctx_sb[qc][:, h, :], cp[:], rbs[:])

                def outproj(qc):
                    ctx = ctx_sb.pop(qc)
                    for ss in range(4):
                        for ec in range(4):
                            op = pop.tile([128, 512], F32, tag="op")
                            for h in range(HPC):
                                nc.tensor.matmul(
                                    op[:], ctx[:, h, ss*128:(ss+1)*128],
                                    wo_sb[ec][:, h, :],
                                    start=(h == 0), stop=(h == HPC - 1))
                            o_sb = opool.tile([128, 512], F32, tag="o")
                            if ec % 2 == 0:
                                nc.scalar.copy(o_sb[:], op[:])
                            else:
                                nc.vector.tensor_copy(o_sb[:], op[:])
                            nc.sync.dma_start(
                                out=out[qc*512+ss*128:qc*512+(ss+1)*128,
                                        ec*512:(ec+1)*512],
                                in_=o_sb[:])

                # Pair short (qc=0/1) units with long (qc=3/2) ones so the
                # serial per-unit reciprocal always hides under a long
                # neighbour's matmul stream; interleave score/ctx matmuls
                # kc-by-kc so the PE never throttles to ACT's exp pace.
                units = []
                for h in range(HPC):
                    units += [(0, h), (3, h)]
                for h in range(HPC):
                    units += [(1, h), (2, h)]
                ndone = {qc: 0 for qc in range(nsc)}
                for i in range(len(units) + 2):
                    cur = units[i] if i < len(units) else None
                    prv = units[i - 1] if 0 <= i - 1 < len(units) else None
                    if cur is not None:
                        st[cur] = espool.tile([128, 16, 512], BF16, tag="es",
                                              name=f"es_{i}")
                    if prv is not None:
                        es_p = st[prv]
                        cp = pcd.tile([128, 512], F32, tag="cd")
                        dn = pdn.tile([1, 512], F32, tag="dn")
                    nk_c = 4 * cur[0] + 4 if cur is not None else 0
                    nk_p = 4 * prv[0] + 4 if prv is not None else 0
                    for kc in range(max(nk_c, nk_p)):
                        if kc < nk_c:
                            emit_score_kc(cur, kc)
                        if kc < nk_p:
                            emit_ctx_kc(prv, kc, cp)
                    if prv is not None:
                        emit_dn(prv, dn)
                        st[prv] = (cp, dn)
                        emit_norm(prv)
                    if 0 <= i - 2 < len(units):
                        u2 = units[i - 2]
                        emit_finish(u2)
                        ndone[u2[0]] += 1
                        if ndone[u2[0]] == HPC:
                            outproj(u2[0])
    split_excess_waits(nc)
    return nc


_NC_CACHE = {}


def _get_nc(seq):
    if seq not in _NC_CACHE:
        _NC_CACHE[seq] = build_nc(seq)
    return _NC_CACHE[seq]


def make_in_maps(x, Wq, bq, Wk, bk, Wv, bv, Wo, bo, seq=S, nb=B):
    f32 = np.float32
    in_maps = []
    for c in range(NCORES):
        b = c // 2
        half = c % 2
        sl = slice(half * MLOC, (half + 1) * MLOC)
        in_maps.append({
            "xt": np.ascontiguousarray(x[b].T).astype(BF16_NP),
            "wqt": np.ascontiguousarray(Wq[sl, :].T).astype(BF16_NP),
            "wkt": np.ascontiguousarray(Wk[sl, :].T).astype(BF16_NP),
            "wvt": np.ascontiguousarray(Wv[sl, :].T).astype(BF16_NP),
            "wot": np.ascontiguousarray(Wo[:, sl].T).astype(BF16_NP),
            "bqt": np.ascontiguousarray(bq[sl].reshape(HPC, DK).T, dtype=f32),
            "bkt": np.ascontiguousarray(bk[sl].reshape(HPC, DK).T, dtype=f32),
            "ones": np.ones((128, 128), dtype=f32),
        })
    return in_maps


def run(inputs, trace=False, trace_kwargs=None):
    x = np.asarray(inputs["x"], dtype=np.float32)
    nb, seq, d = x.shape
    nc = _get_nc(seq)
    Wo = np.asarray(inputs["Wo"], dtype=np.float32)
    bv = np.asarray(inputs["bv"], dtype=np.float32)
    bo = np.asarray(inputs["bo"], dtype=np.float32)
    in_maps = make_in_maps(
        x, np.asarray(inputs["Wq"]), np.asarray(inputs["bq"]),
        np.asarray(inputs["Wk"]), np.asarray(inputs["bk"]),
        np.asarray(inputs["Wv"]), bv, Wo, bo, seq=seq, nb=nb)
    res = run_bass_kernel_spmd(nc, in_maps, list(range(NCORES)), trace=trace,
                               **(trace_kwargs or {}))
    # V-bias folds out of the device program: ctx = attn@V + bv (rows of
    # attn sum to 1), so its out-projection contribution is bv @ Wo.T.
    bo_eff = bo + Wo @ bv
    out = np.empty((nb, seq, d), dtype=np.float32)
    for b in range(nb):
        out[b] = res.results[2*b]["out"] + res.results[2*b+1]["out"] + bo_eff
    return out, res


def kernel(**inputs):
    out, _ = run(inputs, trace=False)
    return out
